# revision 19
# baseline (speedup 1.0000x reference)
"""Trainium2 Bass kernel for a 2-layer GCN encoder + global mean pool.

Problem: x[100000,128] f32, edge_index[2,1600000] i64, batch[100000] i64
(sorted), W1/b1/W2/b2. Two GCNConv layers (symmetric deg^-1/2 norm, self
loops, relu) then mean-pool over 512 graphs -> [512, 128] f32.

Strategy (8 NeuronCores, data-parallel over graphs):
- Nodes partitioned by graph id into 8 contiguous shards (batch is sorted);
  each core owns the edges whose *destination* lands in its shard.
- Algebraic rewrite: aggregate-then-transform.  For each layer,
      h' = relu( (A_hat @ h) @ W + b ),   A_hat = D^-1/2 (A+I) D^-1/2
  so the sparse aggregation runs on raw 128-dim features and the dense
  matmul with W happens per 128-node window afterwards.
- Self loops never touch the gather path: each destination window starts
  its PSUM accumulation with a diagonal matmul
      PSUM_w  = X_own[w]^T-style (lhsT=X_own_window, rhs=diag(1/deg))
  where the window's own rows come from a per-core contiguous upload
  (layer 1) or from the SBUF-resident h1 window tiles (layer 2).
- Sparse aggregation per core: per 128-edge block, dma_gather fetches the
  128 source rows (bf16, 256B each) from the feature table; a one-hot
  selection matrix M (built on the Vector engine from precomputed dst-slot
  and edge-weight columns) scatters them on the Tensor engine:
      PSUM[f, s] += G_blk.T @ M_blk
  accumulated over all blocks of a 128-node destination window.
- dma_gather uses int16 indices (<=32768-row "quarter" table slices).
  Edges are bucketed by (window-group, quarter, window); gather calls
  never cross a (group, quarter, window) segment so each core's unused
  tail of a segment is filled with -1 indices, which the gather ucode
  trims at 128-index granularity: descriptor generation (the kernel's
  bottleneck, ~21ns/idx on one Q7 pair; calls are spread over the 4
  SWDGE queue-pairs by expected load) only pays for this core's own
  edges, not the SPMD max across cores. The trimmed per-call count is
  fed to each gather through a Pool register (8 loaded per reg_load)
  so the decode-side descriptor-ring reservation matches the ucode's
  value trim.
- The finished PSUM window (= Z^T) feeds the dense W matmul directly (its
  transposed layout is exactly the lhsT the Tensor engine wants); bias is
  added via a K=1 matmul with a ones row; relu on the Scalar engine.
- Between layers one 8-core AllGather shares the per-shard h1 table (bf16).
- Mean pool: the same one-hot matmul trick keyed on local graph id into a
  single resident PSUM bank, then a reciprocal-count scale.

Host-side preprocessing is purely structural (index sorting, degree counts,
normalization coefficients derived from the graph topology); all
feature/weight compute runs on device.
"""

import math
import os

import numpy as np
import ml_dtypes

import concourse.bass as bass
import concourse.bacc as bacc
import concourse.mybir as mybir
import concourse.tile as tile
from concourse.bass_utils import run_bass_kernel_spmd

P = 128
C = 8               # cores
G = 512             # graphs
GPC = G // C        # graphs per core
F = 128             # feature dim (in = hid = out)
WG = 4              # windows per PSUM-resident group
QROWS = 32768       # rows per int16-addressable table slice
NQ = 4              # table quarters
CALLBLK = 8         # max edge blocks per dma_gather call (1024 idx;
                    # larger single calls overflow the SWDGE desc ring)

bf16 = mybir.dt.bfloat16
f32 = mybir.dt.float32
i16 = mybir.dt.int16

BF = ml_dtypes.bfloat16


def _preprocess(x, edge_index, batch):
    """Structural preprocessing: shard nodes by graph, bucket/pad edges by
    (window-group, src-quarter, dst-window), compute GCN norm weights."""
    N = x.shape[0]
    src = np.asarray(edge_index[0], dtype=np.int64)
    dst = np.asarray(edge_index[1], dtype=np.int64)
    batch = np.asarray(batch, dtype=np.int64)

    node_start = np.searchsorted(batch, np.arange(C + 1) * GPC).astype(np.int64)
    nk = np.diff(node_start)
    NODE_PAD = int(math.ceil(nk.max() / P) * P)
    NW = NODE_PAD // P
    TOT = C * NODE_PAD
    assert TOT <= NQ * QROWS
    NG = (NW + WG - 1) // WG

    core_of = (batch // GPC).astype(np.int64)
    row = (np.arange(N) - node_start[core_of] + core_of * NODE_PAD).astype(np.int64)

    deg = np.bincount(dst, minlength=N).astype(np.float64) + 1.0
    dis = 1.0 / np.sqrt(deg)

    # real edges only; self loops are handled by per-window diagonal matmuls
    ew = (dis[src] * dis[dst]).astype(np.float32)

    ecore = core_of[dst]
    eld = dst - node_start[ecore]
    ewin = eld >> 7
    eslot = (eld & 127).astype(np.float32)
    esrcrow = row[src]
    eq = (esrcrow // QROWS).astype(np.int64)      # source quarter
    eloc = (esrcrow % QROWS).astype(np.int64)     # quarter-local row
    egrp = ewin // WG

    # segment id in (core, group, quarter, window) order
    seg = ((ecore * NG + egrp) * NQ + eq) * NW + ewin
    NSEG = C * NG * NQ * NW
    counts = np.bincount(seg, minlength=NSEG)
    cnt4 = counts.reshape(C, NG, NQ, NW)
    # SPMD-common block counts per (group, quarter, window): max over cores
    BWS = np.ceil(cnt4.max(axis=0) / P).astype(np.int64)      # [NG, NQ, NW]
    for g in range(NG):
        mask = np.zeros(NW, bool)
        mask[g * WG:(g + 1) * WG] = True
        BWS[g, :, ~mask] = 0
    NBLK = int(BWS.sum())

    # block/segment offsets in (g, q, w) order
    seg_order = []          # (g, q, w, block_start, nblocks)
    seg_start = np.zeros((NG, NQ, NW), np.int64)
    acc = 0
    for g in range(NG):
        for q in range(NQ):
            for w in range(g * WG, min((g + 1) * WG, NW)):
                seg_start[g, q, w] = acc
                nb = int(BWS[g, q, w])
                if nb:
                    seg_order.append((g, q, w, acc, nb))
                acc += nb
    assert acc == NBLK

    # scatter edges into the padded per-core layout; unfilled slots keep
    # idx=-1 (runtime-trimmed by the gather ucode) and weight 0
    order = np.argsort(seg, kind="stable")
    seg_sorted = seg[order]
    grp_excl = np.concatenate([[0], np.cumsum(counts)[:-1]])
    pos = np.arange(order.size) - grp_excl[seg_sorted]
    es = order
    dest = (ecore[es] * (NBLK * P)
            + seg_start[egrp[es], eq[es], ewin[es]] * P + pos)

    idx_arr = np.full(C * NBLK * P, -1, np.int16)   # quarter-local src row
    slot_arr = np.zeros(C * NBLK * P, np.float32)
    w_arr = np.zeros(C * NBLK * P, np.float32)
    idx_arr[dest] = eloc[es].astype(np.int16)
    slot_arr[dest] = eslot[es]
    w_arr[dest] = ew[es]

    # per-core uploads
    # idx: wrapped [16, NBLK*8] (logical i at [i%16, i//16]), replicated to
    # 128 partitions (the gather ucode's per-Q7-core channel groups all read
    # the same wrap)
    idx_pc = np.ascontiguousarray(
        idx_arr.reshape(C, NBLK * P // 16, 16).transpose(0, 2, 1))
    idx_pc = np.ascontiguousarray(np.tile(idx_pc, (1, 8, 1)))
    slot_pc = np.ascontiguousarray(slot_arr.reshape(C, NBLK, P).transpose(0, 2, 1))
    w_pc = np.ascontiguousarray(w_arr.reshape(C, NBLK, P).transpose(0, 2, 1))

    # node feature table, padded/bf16; per-core own-shard rows
    xt = np.zeros((TOT, F), BF)
    xt[row] = np.asarray(x, np.float32).astype(BF)
    xown_pc = np.ascontiguousarray(xt.reshape(C, NODE_PAD, F))

    # per-window self-loop weights: 1/deg for own nodes, 0 for pad rows
    selfw = np.zeros((C, NODE_PAD), np.float32)
    for c in range(C):
        nn = int(nk[c])
        selfw[c, :nn] = (1.0 / deg[node_start[c]:node_start[c + 1]]).astype(
            np.float32)
    selfw_pc = np.ascontiguousarray(selfw.reshape(C, NW, P).transpose(0, 2, 1))

    # static schedule: per block -> window, last-of-window; gather calls are
    # chunks of <= CALLBLK blocks that never cross a (g, q, w) segment (so
    # each core's unused tail of every call is trailing -1 indices)
    blk_win = np.zeros(NBLK, np.int64)
    last_blk = {}
    for (g, q, w, b0, nb) in seg_order:
        blk_win[b0:b0 + nb] = w
        last_blk[w] = b0 + nb - 1
    blk_last = np.zeros(NBLK, bool)
    for w, b in last_blk.items():
        blk_last[b] = True

    calls = []   # (b0, nb, quarter, group)
    for (g, q, w, b0, nb) in seg_order:
        b = b0
        while b < b0 + nb:
            nbc = min(CALLBLK, b0 + nb - b)
            calls.append((b, nbc, q, g))
            b += nbc

    # per-core per-call valid-index counts: the gather ucode trims trailing
    # -1 indices by value; the decode side sizes its descriptor-ring
    # reservation from num_idxs_reg. Both must agree, so the register value
    # for each call is this core's exact count of real indices in the call.
    seg_of_call = []
    for (g, q, w, b0, nb) in seg_order:
        b = b0
        while b < b0 + nb:
            nbc = min(CALLBLK, b0 + nb - b)
            seg_of_call.append((g, q, w, b0, b, nbc))
            b += nbc
    assert len(seg_of_call) == len(calls)
    call_cnt = np.zeros((C, len(calls)), np.int32)
    for ci, (g, q, w, segb0, b, nbc) in enumerate(seg_of_call):
        own = cnt4[:, g, q, w]                       # [C]
        off = (b - segb0) * P
        call_cnt[:, ci] = np.clip(own - off, 0, nbc * P)

    # balance SWDGE queue-pair load using the mean trimmed count per call
    # (queue k's descriptor work runs on Q7 core pair k; counts are per-core
    # but the queue id is SPMD-shared, so balance the expectation)
    mean_cnt = call_cnt.mean(axis=0)
    qload = np.zeros(4)
    call_q = np.zeros(len(calls), np.int64)
    for ci in range(len(calls)):
        k = int(np.argmin(qload))
        call_q[ci] = k
        qload[k] += mean_cnt[ci] + 150.0   # + per-call fixed overhead proxy

    # pad the per-call count table to a multiple of 8 so the kernel can load
    # 8 count registers per instruction
    NCALLS8 = (len(calls) + 7) // 8 * 8
    call_cnt8 = np.zeros((C, NCALLS8), np.int32)
    call_cnt8[:, :len(calls)] = call_cnt

    # pooling metadata
    batloc = np.full((C, NODE_PAD), -1.0, np.float32)
    for c in range(C):
        nn = int(nk[c])
        batloc[c, :nn] = (batch[node_start[c]:node_start[c + 1]] - c * GPC).astype(
            np.float32)
    batloc_pc = np.ascontiguousarray(batloc.reshape(C, NW, P).transpose(0, 2, 1))

    gcnt = np.bincount(batch, minlength=G).astype(np.float32)
    counts_pc = np.ones((C, P, 1), np.float32)
    counts_pc[:, :GPC, 0] = gcnt.reshape(C, GPC)

    return dict(
        NODE_PAD=NODE_PAD, NW=NW, NG=NG, TOT=TOT, NBLK=NBLK,
        blk_win=blk_win, blk_last=blk_last, calls=calls, call_cnt=call_cnt8,
        call_q=call_q,
        idx_pc=idx_pc, slot_pc=slot_pc, w_pc=w_pc, xt=xt, xown_pc=xown_pc,
        selfw_pc=selfw_pc, batloc_pc=batloc_pc, counts_pc=counts_pc,
    )


def _build_nc(pre):
    NW = pre["NW"]
    NG = pre["NG"]
    NBLK = pre["NBLK"]
    TOT = pre["TOT"]
    NODE_PAD = pre["NODE_PAD"]
    blk_win = pre["blk_win"]
    blk_last = pre["blk_last"]
    calls = pre["calls"]

    _nq = int(os.environ.get("KERNEL_NQUEUES", "4"))
    nc = bacc.Bacc(None, num_devices=C, num_swdge_queues=_nq)
    NCALLS = len(calls)
    NCALLS8 = pre["call_cnt"].shape[1]
    call_q = pre["call_q"]

    xt_d = nc.dram_tensor("xt", [TOT, F], bf16, kind="ExternalInput")
    xown_d = nc.dram_tensor("xown", [NODE_PAD, F], bf16, kind="ExternalInput")
    idx_d = nc.dram_tensor("eidx", [128, NBLK * 8], i16, kind="ExternalInput")
    slot_d = nc.dram_tensor("eslot", [P, NBLK], f32, kind="ExternalInput")
    ew_d = nc.dram_tensor("ew", [P, NBLK], f32, kind="ExternalInput")
    iota_d = nc.dram_tensor("iota", [P, P], bf16, kind="ExternalInput")
    iotac_d = nc.dram_tensor("iotac", [P, 1], f32, kind="ExternalInput")
    ones_d = nc.dram_tensor("ones", [1, P], bf16, kind="ExternalInput")
    w1_d = nc.dram_tensor("w1", [F, F], bf16, kind="ExternalInput")
    w2_d = nc.dram_tensor("w2", [F, F], bf16, kind="ExternalInput")
    b1_d = nc.dram_tensor("b1", [1, F], bf16, kind="ExternalInput")
    b2_d = nc.dram_tensor("b2", [1, F], bf16, kind="ExternalInput")
    selfw_d = nc.dram_tensor("selfw", [P, NW], f32, kind="ExternalInput")
    batloc_d = nc.dram_tensor("batloc", [P, NW], f32, kind="ExternalInput")
    cnts_d = nc.dram_tensor("cnts", [P, 1], f32, kind="ExternalInput")
    gcall_d = nc.dram_tensor("gcall", [1, NCALLS8], mybir.dt.int32,
                             kind="ExternalInput")
    out_d = nc.dram_tensor("out", [GPC, F], f32, kind="ExternalOutput")

    # group the call list by window-group for group-major emission
    calls_by_g = [[] for _ in range(NG)]
    for ci, (b0, nb, q, g) in enumerate(calls):
        calls_by_g[g].append((b0, nb, q, ci))

    with tile.TileContext(nc) as tc:
        with (
            tc.tile_pool(name="const", bufs=1) as cpool,
            tc.tile_pool(name="gbuf", bufs=8) as gpool,
            tc.tile_pool(name="xo", bufs=8) as xopool,
            tc.tile_pool(name="mt", bufs=8) as mtpool,
            tc.tile_pool(name="zt", bufs=2) as ztpool,
            tc.tile_pool(name="h1own", bufs=1) as h1pool,
            tc.tile_pool(name="hsb", bufs=2) as hpool,
            tc.tile_pool(name="osb", bufs=2) as opool,
            tc.tile_pool(name="psw", bufs=WG + 1, space="PSUM") as pswpool,
            tc.tile_pool(name="psh", bufs=2, space="PSUM") as pshpool,
            tc.tile_pool(name="psp", bufs=1, space="PSUM") as psppool,
            tc.tile_pool(name="dram", bufs=1, space="DRAM") as dpool,
        ):
            # --- constants ---
            idx_sb = cpool.tile([128, NBLK * 8], i16)
            nc.sync.dma_start(out=idx_sb[:], in_=idx_d[:])
            slot_sb = cpool.tile([P, NBLK], f32)
            nc.sync.dma_start(out=slot_sb[:], in_=slot_d[:])
            ew_sb = cpool.tile([P, NBLK], f32)
            nc.sync.dma_start(out=ew_sb[:], in_=ew_d[:])
            iota_sb = cpool.tile([P, P], bf16)
            nc.sync.dma_start(out=iota_sb[:], in_=iota_d[:])
            iotac_sb = cpool.tile([P, 1], f32)
            nc.sync.dma_start(out=iotac_sb[:], in_=iotac_d[:])
            ones_sb = cpool.tile([1, P], bf16)
            nc.sync.dma_start(out=ones_sb[:], in_=ones_d[:])
            w1_sb = cpool.tile([F, F], bf16)
            nc.sync.dma_start(out=w1_sb[:], in_=w1_d[:])
            w2_sb = cpool.tile([F, F], bf16)
            nc.sync.dma_start(out=w2_sb[:], in_=w2_d[:])
            b1_sb = cpool.tile([1, F], bf16)
            nc.sync.dma_start(out=b1_sb[:], in_=b1_d[:])
            b2_sb = cpool.tile([1, F], bf16)
            nc.sync.dma_start(out=b2_sb[:], in_=b2_d[:])
            selfw_sb = cpool.tile([P, NW], f32)
            nc.sync.dma_start(out=selfw_sb[:], in_=selfw_d[:])
            batloc_sb = cpool.tile([P, NW], f32)
            nc.sync.dma_start(out=batloc_sb[:], in_=batloc_d[:])
            cnts_sb = cpool.tile([P, 1], f32)
            nc.sync.dma_start(out=cnts_sb[:], in_=cnts_d[:])
            gcall_sb = cpool.tile([1, NCALLS8], mybir.dt.int32)
            nc.sync.dma_start(out=gcall_sb[:], in_=gcall_d[:])

            # Funnel const-tile deps through the Vector engine (the ISA has a
            # small per-instruction sync-wait budget; same-engine ordering is
            # free).
            scratch = cpool.tile([P, 1], f32)
            for t in (slot_sb, ew_sb, iota_sb, iotac_sb, w1_sb, w2_sb,
                      selfw_sb, batloc_sb, cnts_sb):
                nc.vector.reduce_sum(out=scratch[:], in_=t[:],
                                     axis=mybir.AxisListType.X)
            for t in (ones_sb, b1_sb, b2_sb):
                nc.vector.reduce_sum(out=scratch[:1, :], in_=t[:],
                                     axis=mybir.AxisListType.X)

            # gather tiles may keep stale data in runtime-trimmed blocks
            # (weight-0 one-hots zero them out, but NaN*0=NaN) — zero every
            # pool slot once so stale always means "finite"
            for _ in range(8):
                gz = gpool.tile([P, CALLBLK, P], bf16, tag="g")
                nc.vector.memset(gz[:], 0.0)

            h1_shard = dpool.tile([NODE_PAD, F], bf16)
            h1_table = dpool.tile([TOT, F], bf16, addr_space="Shared")

            # registers feeding each gather's runtime index count (Pool decode
            # reads it for descriptor-ring accounting; must match the value
            # trim of the trailing -1 indices). 8 regs loaded per instruction.
            cnt_regs = [nc.gpsimd.alloc_register(f"gcnt_reg{i}")
                        for i in range(8)]

            # layer-1 outputs stay resident for layer-2's diagonal matmuls
            h1own = [h1pool.tile([P, F], bf16, name=f"h1w{w}")
                     for w in range(NW)]

            pool_ps = psppool.tile([P, F], f32)

            for layer in range(2):
                table = xt_d if layer == 0 else h1_table
                wmat_sb = w1_sb if layer == 0 else w2_sb
                b_sb = b1_sb if layer == 0 else b2_sb

                ps_tiles = {}
                for g in range(NG):
                    ws = range(g * WG, min((g + 1) * WG, NW))
                    for w in ws:
                        ps_w = pswpool.tile([P, P], f32, tag="psw",
                                            name=f"psw{w % 8}")
                        ps_tiles[w] = ps_w
                        if layer == 0:
                            own = xopool.tile([P, F], bf16, tag="xo")
                            nc.sync.dma_start(
                                out=own[:], in_=xown_d[w * P:(w + 1) * P, :])
                        else:
                            own = h1own[w]
                        md = mtpool.tile([P, P], bf16, tag="mt")
                        nc.vector.tensor_scalar(
                            out=md[:],
                            in0=iota_sb[:],
                            scalar1=iotac_sb[:, 0:1],
                            scalar2=selfw_sb[:, w:w + 1],
                            op0=mybir.AluOpType.is_equal,
                            op1=mybir.AluOpType.mult,
                        )
                        nc.tensor.matmul(
                            ps_w[:], lhsT=own[:], rhs=md[:],
                            start=True, stop=False,
                        )

                    for (b0, nbk, q, ci) in calls_by_g[g]:
                        if ci % 8 == 0:
                            nc.gpsimd.reg_load(
                                cnt_regs, gcall_sb[0:1, ci:ci + 8])
                        g_t = gpool.tile([P, CALLBLK, P], bf16, tag="g")
                        nc.gpsimd.dma_gather(
                            out_ap=g_t[:, :nbk, :],
                            in_ap=table[q * QROWS:min((q + 1) * QROWS, TOT), :],
                            idxs_ap=idx_sb[:, b0 * 8:(b0 + nbk) * 8],
                            num_idxs=nbk * P,
                            num_idxs_reg=cnt_regs[ci % 8],
                            elem_size=F,
                            queue_num=(int(call_q[ci]) % _nq),
                        )
                        for j in range(nbk):
                            blk = b0 + j
                            w = int(blk_win[blk])
                            ps_w = ps_tiles[w]
                            mt = mtpool.tile([P, P], bf16, tag="mt")
                            nc.vector.tensor_scalar(
                                out=mt[:],
                                in0=iota_sb[:],
                                scalar1=slot_sb[:, blk:blk + 1],
                                scalar2=ew_sb[:, blk:blk + 1],
                                op0=mybir.AluOpType.is_equal,
                                op1=mybir.AluOpType.mult,
                            )
                            nc.tensor.matmul(
                                ps_w[:],
                                lhsT=g_t[:, j, :],
                                rhs=mt[:],
                                start=False,
                                stop=bool(blk_last[blk]),
                            )
                            if blk_last[blk]:
                                # ---- dense part for finished window w ----
                                zt = ztpool.tile([P, P], bf16, tag="zt")
                                nc.vector.tensor_copy(out=zt[:], in_=ps_w[:])
                                del ps_tiles[w]
                                ps_h = pshpool.tile([P, F], f32, tag="psh")
                                nc.tensor.matmul(
                                    ps_h[:], lhsT=zt[:], rhs=wmat_sb[:],
                                    start=True, stop=False,
                                )
                                nc.tensor.matmul(
                                    ps_h[:], lhsT=ones_sb[:], rhs=b_sb[:],
                                    start=False, stop=True,
                                )
                                if layer == 0:
                                    nc.scalar.activation(
                                        out=h1own[w][:], in_=ps_h[:],
                                        func=mybir.ActivationFunctionType.Relu,
                                    )
                                    nc.sync.dma_start(
                                        out=h1_shard[w * P:(w + 1) * P, :],
                                        in_=h1own[w][:],
                                    )
                                else:
                                    h_sb = hpool.tile([P, F], bf16, tag="h")
                                    nc.scalar.activation(
                                        out=h_sb[:], in_=ps_h[:],
                                        func=mybir.ActivationFunctionType.Relu,
                                    )
                                    mb = mtpool.tile([P, P], bf16, tag="mt")
                                    nc.vector.tensor_scalar(
                                        out=mb[:],
                                        in0=iota_sb[:],
                                        scalar1=batloc_sb[:, w:w + 1],
                                        scalar2=None,
                                        op0=mybir.AluOpType.is_equal,
                                    )
                                    nc.tensor.matmul(
                                        pool_ps[:],
                                        lhsT=mb[:],
                                        rhs=h_sb[:],
                                        start=(w == 0),
                                        stop=(w == NW - 1),
                                    )

                if layer == 0:
                    nc.gpsimd.collective_compute(
                        "AllGather",
                        mybir.AluOpType.bypass,
                        replica_groups=[list(range(C))],
                        ins=[h1_shard[:]],
                        outs=[h1_table[:]],
                    )

            # ---- finalize pool: divide by counts ----
            rec_sb = opool.tile([P, 1], f32, tag="rec")
            nc.vector.reciprocal(out=rec_sb[:], in_=cnts_sb[:])
            out_sb = opool.tile([P, F], f32, tag="os")
            nc.vector.tensor_scalar(
                out=out_sb[:],
                in0=pool_ps[:],
                scalar1=rec_sb[:, 0:1],
                scalar2=None,
                op0=mybir.AluOpType.mult,
            )
            nc.sync.dma_start(out=out_d[:], in_=out_sb[0:GPC, :])

    nc.compile()
    return nc


def kernel(x, edge_index, batch, W1, b1, W2, b2):
    x = np.asarray(x, np.float32)
    pre = _preprocess(x, edge_index, batch)

    iota = np.broadcast_to(np.arange(P, dtype=np.float32), (P, P)).astype(BF)
    iotac = np.arange(P, dtype=np.float32).reshape(P, 1)
    ones = np.ones((1, P), BF)
    w1b = np.asarray(W1, np.float32).astype(BF)
    w2b = np.asarray(W2, np.float32).astype(BF)
    b1b = np.asarray(b1, np.float32).reshape(1, F).astype(BF)
    b2b = np.asarray(b2, np.float32).reshape(1, F).astype(BF)

    in_maps = []
    for c in range(C):
        in_maps.append({
            "xt": pre["xt"],
            "xown": pre["xown_pc"][c],
            "eidx": pre["idx_pc"][c],
            "eslot": pre["slot_pc"][c],
            "ew": pre["w_pc"][c],
            "iota": iota,
            "iotac": iotac,
            "ones": ones,
            "w1": w1b,
            "w2": w2b,
            "b1": b1b,
            "b2": b2b,
            "selfw": pre["selfw_pc"][c],
            "batloc": pre["batloc_pc"][c],
            "cnts": pre["counts_pc"][c],
            "gcall": pre["call_cnt"][c].reshape(1, -1),
        })

    nc = _build_nc(pre)
    res = run_bass_kernel_spmd(nc, in_maps, core_ids=list(range(C)))
    out = np.concatenate([res.results[c]["out"] for c in range(C)], axis=0)
    return out.astype(np.float32)
